# revision 7
# baseline (speedup 1.0000x reference)
"""ConceptCLIP loss kernel for 8x Trainium2 NeuronCores (Bass/Tile), v2.

Strategy (data-parallel over the image batch axis m):
  - Each core owns 16 of the 128 images; concepts/text features are
    replicated. Concepts are host-packed (only w < counts[v] kept) and the
    concept L2 norm is deferred into the host-built gather matrix G.
  - The big patch x concept similarity matmul runs in fp8e4 with
    MatmulPerfMode.DoubleRow (K=256 per instruction, 2x bf16 throughput).
    fp8 error analysis: cosine rms error ~3e-3 against values +-0.15 ->
    rc_loss relative error ~0.2%, far inside the 2e-2 gate.
  - Patches are shipped fp8 (n,d) flat (3136 rows); per 128-row block:
    ACT square+accum -> sqrt -> DVE recip -> DVE scale to bf16, then an
    XBAR dma_start_transpose lands the normalized block directly in the
    (d-chunk, n) rhs layout (bf16 staging), cast-copied to fp8. The PE
    never runs patch transposes.
  - Main loop: one pass per 8-image half, per 128-concept chunk c: 3
    DoubleRow k-pairs x 4 PSUM-bank chains of 392 cols (2 images each),
    then a single 4D DVE reduce_max -> maxcol bf16.
  - S = G_eff^T @ maxcol in bf16, IT-align on CLS features in bf16,
    softplus losses on device, host sums the per-element losses.
"""

import math
import os
import sys

for _p in ("/opt/trn_rl_repo", "/root/.axon_site/_ro/trn_rl_repo"):
    if os.path.isdir(_p) and _p not in sys.path:
        sys.path.insert(0, _p)

import ml_dtypes
import numpy as np

import concourse.tile as tile
from concourse import bacc, mybir
from concourse.bass_utils import run_bass_kernel_spmd

BF16 = ml_dtypes.bfloat16
FP8 = ml_dtypes.float8_e4m3

N_CORES = 8
B, NPATCH, D, W = 128, 196, 768, 32
M_PER = B // N_CORES          # 16 images per core
KC = D // 128                 # 6 contraction chunks
NPF = M_PER * NPATCH          # 3136 flat patch columns per core
NBLK = math.ceil(NPF / 128)   # 25 prep blocks (24x128 + 64)
NPAD = NBLK * 128             # 3200 padded patch rows
HALF = NPF // 2               # 1568 columns per main-loop pass
CHW = 2 * NPATCH              # 392-column chains (2 images per PSUM bank)

F32 = mybir.dt.float32
BF = mybir.dt.bfloat16
F8 = mybir.dt.float8e4
AX = mybir.AxisListType
AF = mybir.ActivationFunctionType
PM = mybir.MatmulPerfMode

_cache = {}


def _build(C, t, bias):
    """Build + compile the per-core Bass program. C = number of 128-row packed
    concept chunks; t/bias are compile-time scalar constants."""
    P = C * 128
    nc = bacc.Bacc("TRN2", target_bir_lowering=False, debug=False,
                   num_devices=N_CORES)

    d_patches = nc.dram_tensor("patches", (NPAD, D), BF, kind="ExternalInput")
    d_cT = nc.dram_tensor("cT", (KC, 128, P), F8, kind="ExternalInput")
    d_cnat = nc.dram_tensor("cnat", (P, D), F8, kind="ExternalInput")
    d_GT = nc.dram_tensor("GT", (C, 128, B), F32, kind="ExternalInput")
    d_img = nc.dram_tensor("img", (M_PER, D), BF, kind="ExternalInput")
    d_txt = nc.dram_tensor("txt", (B, D), BF, kind="ExternalInput")
    d_sign = nc.dram_tensor("signneg", (B, M_PER), F32, kind="ExternalInput")
    d_ident = nc.dram_tensor("ident", (128, 128), BF, kind="ExternalInput")
    d_rc = nc.dram_tensor("rc_el", (B, M_PER), F32, kind="ExternalOutput")
    d_it = nc.dram_tensor("it_el", (B, M_PER), F32, kind="ExternalOutput")

    with tile.TileContext(nc) as tc:
        with (
            tc.tile_pool(name="consts", bufs=1) as consts,
            tc.tile_pool(name="work", bufs=3) as work,
            tc.tile_pool(name="small", bufs=4) as small,
            tc.tile_pool(name="psum", bufs=2, space="PSUM") as psum,
        ):
            # preload the ACT tables needed during prep while DMAs fill
            warm = small.tile([1, 1], F32, tag="warm")
            nc.vector.memset(warm[:], 1.0)
            nc.scalar.activation(out=warm[:], in_=warm[:], func=AF.Square)
            nc.scalar.sqrt(warm[:], warm[:])
            nc.scalar.activation(out=warm[:], in_=warm[:], func=AF.Copy,
                                 bias=0.0, scale=1.0)

            # big SBUF residents
            rhs = consts.tile([128, KC, NPAD, 2], F8, tag="rhs")    # patchesT (stride-2)
            cT = consts.tile([128, KC, P], F8, tag="cT")            # conceptsT
            cnat = consts.tile([128, C, D], F8, tag="cnat")
            GT = consts.tile([128, C, B], F32, tag="GT")
            GTb = consts.tile([128, C, B], BF, tag="GTb")
            maxcol = consts.tile([128, C, M_PER], BF, tag="maxcol")
            rnorm = consts.tile([128, C], F32, tag="rnorm")
            txtT = consts.tile([128, KC, 128], BF, tag="txtT")
            imgT = consts.tile([128, KC, M_PER], BF, tag="imgT")
            yit = consts.tile([B, M_PER], F32, tag="yit")

            # ---------- loads: tiny consts on sync, patches first on gpsimd
            ident = consts.tile([128, 128], BF, tag="ident")
            nc.sync.dma_start(out=ident[:], in_=d_ident.ap())
            ident8 = consts.tile([128, 128], F8, tag="ident8")
            nc.scalar.copy(out=ident8[:], in_=ident[:])
            txt_t = work.tile([128, D], BF, tag="txtld", bufs=1)
            nc.sync.dma_start(out=txt_t[:], in_=d_txt.ap())
            img_t = work.tile([128, D], BF, tag="imgld", bufs=1)
            nc.sync.dma_start(out=img_t[0:M_PER], in_=d_img.ap())
            sign = consts.tile([B, M_PER], F32, tag="sign")
            nc.sync.dma_start(out=sign[:], in_=d_sign.ap())

            # patch loads + cT keep the DMA ring clear for early prep; the
            # cnat/GT loads are dispatched mid-pass-0 (see extras below).
            nats = []
            LOADB = 5  # blocks per load DMA
            for g in range(math.ceil(NBLK / LOADB)):
                b0, b1 = g * LOADB, min((g + 1) * LOADB, NBLK)
                natg = work.tile([128, LOADB, D], BF, tag="natg", bufs=6,
                                 name=f"natg{g}")
                src = d_patches.ap()[b0 * 128:b1 * 128, :].rearrange(
                    "(blk p) d -> p blk d", p=128)
                nc.gpsimd.dma_start(out=natg[:, 0:b1 - b0, :], in_=src)
                nats.append(natg)

            nc.gpsimd.dma_start(out=cT[:], in_=d_cT.ap().rearrange(
                "k p n -> p k n"))
            nc.gpsimd.dma_start(out=cnat[:], in_=d_cnat.ap().rearrange(
                "(c p) d -> p c d", p=128))
            nc.gpsimd.dma_start(out=GT[:], in_=d_GT.ap().rearrange(
                "c p v -> p c v"))

            def block_rinv_bn(nat, nrows):
                # per-patch 1/||row|| via DVE bn_stats (pre-main blocks)
                st = small.tile([128, 2, 6], F32, tag="st", bufs=4)
                nc.vector.bn_stats(out=st[:nrows, 0, :],
                                   in_=nat[:nrows, 0:D // 2])
                nc.vector.bn_stats(out=st[:nrows, 1, :],
                                   in_=nat[:nrows, D // 2:D])
                ag = small.tile([128, 2], F32, tag="ag", bufs=4)
                nc.vector.bn_aggr(out=ag[:nrows], in_=st[:nrows])
                m2 = small.tile([128, 2], F32, tag="m2", bufs=4)
                nc.scalar.activation(out=m2[:nrows, 0:1], in_=ag[:nrows, 0:1],
                                     func=AF.Square)
                nc.vector.tensor_add(m2[:nrows, 1:2], ag[:nrows, 1:2],
                                     m2[:nrows, 0:1])
                rinv = small.tile([128, 1], F32, tag="rinv", bufs=8)
                nc.scalar.activation(out=rinv[:nrows], in_=m2[:nrows, 1:2],
                                     func=AF.Sqrt, scale=float(D))
                nc.vector.reciprocal(rinv[:nrows], rinv[:nrows])
                return rinv

            def block_rinv_act(nat, nrows):
                ssq = small.tile([128, 1], F32, tag="ssq", bufs=8)
                scr = work.tile([128, D], BF, tag="scr", bufs=2)
                nc.scalar.activation(out=scr[:nrows], in_=nat[:nrows],
                                     func=AF.Square, accum_out=ssq[:nrows])
                nc.scalar.sqrt(ssq[:nrows], ssq[:nrows])
                rinv = small.tile([128, 1], F32, tag="rinv", bufs=8)
                nc.vector.reciprocal(rinv[:nrows], ssq[:nrows])
                return rinv

            def prep_block_xbar(b):
                # mid-main path: XBAR transpose, no PSUM / PE use
                nrows = min(128, NPF - b * 128)
                nat = nats[b // LOADB][:, b % LOADB, :]
                rinv = block_rinv_act(nat, nrows)
                nrm = work.tile([128, D], BF, tag="nrm", bufs=6)
                nc.vector.tensor_scalar_mul(nrm[:nrows], nat[:nrows],
                                            rinv[:nrows])
                tp = work.tile([128, KC, 128], BF, tag="tp", bufs=6)
                nc.sync.dma_start_transpose(out=tp[:, :, 0:nrows],
                                            in_=nrm[0:nrows, :])
                nc.scalar.copy(out=rhs[:, :, b * 128:b * 128 + nrows, 0],
                               in_=tp[:, :, 0:nrows])

            def prep_block_pe(b):
                # pre-main path: DVE scale straight to fp8 -> PE fp8
                # transpose (stride-2 PSUM out, a hardware requirement) ->
                # DVE bitcast copy moves the pair-packed bytes to rhs at
                # bf16 2x rate; no ACT cast pass at all.
                nrows = min(128, NPF - b * 128)
                nat = nats[b // LOADB][:, b % LOADB, :]
                rinv = block_rinv_act(nat, nrows)
                nrm8 = work.tile([128, D], F8, tag="nrm8", bufs=6)
                nc.vector.tensor_scalar_mul(nrm8[:nrows], nat[:nrows],
                                            rinv[:nrows])
                ps = psum.tile([128, KC, 128, 2], F8, tag="mm", name="ps_d")
                for k in range(KC):
                    nc.tensor.transpose(ps[:, k, 0:nrows, 0],
                                        nrm8[:nrows, k * 128:(k + 1) * 128],
                                        ident8[:nrows, :nrows])
                nc.vector.tensor_copy(
                    out=rhs[:, :, b * 128:b * 128 + nrows, :].bitcast(BF),
                    in_=ps[:, :, 0:nrows, :].bitcast(BF))

            def concept_chunk(c0, c1):
                # concept norms c0..c1 (deferred into GTb); interleaved into
                # the pass-0 emission so GTb is ready by pass-0 end
                for c in range(c0, min(c1, C)):
                    ssqc = small.tile([128, 1], F32, tag="ssq", bufs=8)
                    scrc = work.tile([128, D], F8, tag="scrc", bufs=2)
                    nc.scalar.activation(out=scrc[:], in_=cnat[:, c, :],
                                         func=AF.Square, accum_out=ssqc[:])
                    nc.scalar.sqrt(ssqc[:], ssqc[:])
                    nc.vector.reciprocal(rnorm[:, c:c + 1], ssqc[:])
                    nc.vector.tensor_scalar_mul(GTb[:, c, :], GT[:, c, :],
                                                rnorm[:, c:c + 1])

            def late_tables():
                nc.scalar.activation(out=warm[:], in_=warm[:], func=AF.Exp)
                nc.scalar.activation(out=warm[:], in_=warm[:], func=AF.Ln)

            # ---------- text / image CLS features (bf16 IT path) ----------
            def rownorm_recip(src_ap, nrows, rinv_ap):
                scr = work.tile([128, D], BF, tag="scrb", bufs=2)
                ssq = small.tile([128, 1], F32, tag="ssq", bufs=8)
                nc.scalar.activation(out=scr[:nrows], in_=src_ap,
                                     func=AF.Square, accum_out=ssq[:nrows])
                nc.scalar.sqrt(ssq[:nrows], ssq[:nrows])
                nc.vector.reciprocal(rinv_ap, ssq[:nrows])

            def norm_transpose(src_tile, nrows, dst, copy_eng):
                rinv = small.tile([128, 1], F32, tag="rinv", bufs=8)
                rownorm_recip(src_tile[:nrows], nrows, rinv[:nrows])
                nrm = work.tile([128, D], BF, tag="nrm", bufs=6)
                nc.vector.tensor_scalar_mul(nrm[:nrows], src_tile[:nrows],
                                            rinv[:nrows])
                ps = psum.tile([128, KC, 128], BF, tag="mm", name="ps_t")
                for k in range(KC):
                    nc.tensor.transpose(ps[:, k, 0:nrows],
                                        nrm[:nrows, k * 128:(k + 1) * 128],
                                        ident[:nrows, :nrows])
                copy_eng(out=dst[:, :, 0:nrows], in_=ps[:, :, 0:nrows])

            # ---------- upfront prep + IT path -----------------------------
            # pass 0 needs columns 0..1567 = blocks 0..12, prepped before the
            # main loop; sumsq/scale engines alternate to balance ACT vs DVE.
            for b in range(5):
                prep_block_pe(b)

            # txt/img/IT emitted early so their PE ops sit ahead of the main
            # matmuls in the queue but their ACT/DVE ops don't delay block 0.
            norm_transpose(txt_t, 128, txtT, nc.vector.tensor_copy)
            norm_transpose(img_t, M_PER, imgT, nc.scalar.copy)
            itps = psum.tile([128, 512], F32, tag="mm", name="itps")
            for k in range(KC):
                nc.tensor.matmul(itps[:, 0:M_PER], lhsT=txtT[:, k, :],
                                 rhs=imgT[:, k, :], start=(k == 0),
                                 stop=(k == KC - 1))
            nc.scalar.activation(out=yit[:], in_=itps[:, 0:M_PER],
                                 func=AF.Copy, bias=float(bias),
                                 scale=float(t))

            for b in range(5, 13):
                prep_block_pe(b)

            def softplus_out(y_ap, d_out):
                el = small.tile([B, M_PER], F32, tag="el", name="el")
                nc.scalar.activation(out=el[:], in_=y_ap, func=AF.Exp)
                nc.vector.tensor_scalar_add(el[:], el[:], 1.0)
                nc.scalar.activation(out=el[:], in_=el[:], func=AF.Ln)
                nc.sync.dma_start(out=d_out.ap(), in_=el[:])

            # ---------- main loop: fp8 DoubleRow patch x concept -----------
            def main_pass(pt, extras=()):
                extras = dict(extras)
                base = pt * HALF
                for c in range(C):
                    ps4 = psum.tile([128, 4, 512], F32, tag="mm", name="ps4")
                    for kp in range(KC // 2):
                        for i in range(4):
                            nc.tensor.matmul(
                                ps4[:, i, 0:CHW],
                                lhsT=cT[:, 2 * kp:2 * kp + 2,
                                        c * 128:(c + 1) * 128],
                                rhs=rhs[:, 2 * kp:2 * kp + 2,
                                        base + i * CHW:base + (i + 1) * CHW,
                                        0],
                                start=(kp == 0), stop=(kp == KC // 2 - 1),
                                perf_mode=PM.DoubleRow)
                    nc.vector.reduce_max(
                        out=maxcol[:, c, pt * 8:pt * 8 + 8].rearrange(
                            "p (i m) -> p i m", i=4),
                        in_=ps4[:, :, 0:CHW].rearrange(
                            "p i (m n) -> p i m n", m=2),
                        axis=AX.X)
                    if c in extras:
                        extras[c]()
                for k in sorted(extras):
                    if k >= C:  # safety: C is data-dependent
                        extras[k]()

            ex0 = {c: (lambda b=13 + c: prep_block_xbar(b))
                   for c in range(12)}
            ex1 = {0: lambda: concept_chunk(0, 6),
                   1: lambda: concept_chunk(6, 12),
                   2: lambda: concept_chunk(12, C),
                   3: late_tables,
                   4: lambda: (nc.vector.tensor_mul(yit[:], yit[:], sign[:]),
                               softplus_out(yit[:], d_it))}
            main_pass(0, ex0)
            main_pass(1, ex1)

            # ---------- S = G_eff^T @ maxcol (bf16), losses ----------------
            sps = psum.tile([128, 512], F32, tag="mm", name="sps")
            for c in range(C):
                nc.tensor.matmul(sps[:, 0:M_PER],
                                 lhsT=GTb[:, c, :],
                                 rhs=maxcol[:, c, :], start=(c == 0),
                                 stop=(c == C - 1))

            yrc = small.tile([B, M_PER], F32, tag="y")
            nc.scalar.activation(out=yrc[:], in_=sps[:, 0:M_PER],
                                 func=AF.Copy, bias=float(bias),
                                 scale=float(t))
            nc.vector.tensor_mul(yrc[:], yrc[:], sign[:])
            softplus_out(yrc[:], d_rc)

    nc.compile()
    return nc


def _install_trace_hook():
    """Register the axon NTFF profiling hook (missing from this image) so
    run_bass_kernel_spmd(trace=True) can capture HW exec time."""
    import contextlib
    import ctypes
    import types

    import concourse.bass_utils as bu

    if "antenv.axon_hooks" in sys.modules:
        return
    so_path = "/opt/axon/libaxon_pjrt.so"

    def _make_hook():
        lib = ctypes.CDLL(so_path)
        if not hasattr(lib, "axon_start_nrt_profile"):
            return None
        lib.axon_start_nrt_profile.argtypes = [ctypes.POINTER(ctypes.c_int64),
                                               ctypes.c_size_t]
        lib.axon_start_nrt_profile.restype = ctypes.c_int64
        lib.axon_stop_nrt_profile.argtypes = [ctypes.c_char_p]
        lib.axon_stop_nrt_profile.restype = ctypes.c_int64

        @contextlib.contextmanager
        def _hook(output_dir, device_ids):
            import jax
            jax.devices()
            if device_ids:
                ids = (ctypes.c_int64 * len(device_ids))(*device_ids)
                rc = lib.axon_start_nrt_profile(ids, len(device_ids))
            else:
                rc = lib.axon_start_nrt_profile(None, 0)
            if rc != 0:
                raise RuntimeError(f"axon_start_nrt_profile rc={rc}")
            try:
                yield
            finally:
                n = lib.axon_stop_nrt_profile(str(output_dir).encode())
                print(f"profile: {n} file(s) written to {output_dir}",
                      file=sys.stderr)

        return _hook

    mod = types.ModuleType("antenv.axon_hooks")
    mod.get_axon_ntff_profile_hook = _make_hook
    sys.modules["antenv.axon_hooks"] = mod
    bu.upload_artifacts = lambda tmpdir: tmpdir  # no S3 in this container


def _prepare(inputs):
    image_features = np.asarray(inputs["image_features"], np.float32)
    text_features = np.asarray(inputs["text_features"], np.float32)
    image_token_features = np.asarray(inputs["image_token_features"], np.float32)
    concept_text_features = np.asarray(inputs["concept_text_features"], np.float32)
    counts = np.asarray(inputs["concept_counts"]).astype(np.int64)
    t = float(np.exp(np.clip(np.float32(inputs["logit_scale"]), -10.0, 10.0)))
    bias = float(np.float32(inputs["logit_bias"]))

    # pack concepts: keep only w < counts[v]; pad rows with ones (zero weight)
    vidx = np.repeat(np.arange(B), counts)
    widx = np.concatenate([np.arange(c) for c in counts])
    P = len(vidx)
    C = math.ceil(P / 128)
    Ppad = C * 128
    cnat = np.ones((Ppad, D), np.float32)
    cnat[:P] = concept_text_features[vidx, widx]
    cnat8 = cnat.astype(FP8)
    cT = np.ascontiguousarray(cnat8.T).reshape(KC, 128, Ppad)

    G = np.zeros((Ppad, B), np.float32)
    G[np.arange(P), vidx] = 1.0 / counts[vidx]
    GT = G.reshape(C, 128, B)

    txt_bf = text_features.astype(BF16)
    ident = np.eye(128, dtype=BF16)

    in_maps = []
    for core in range(N_CORES):
        s = slice(core * M_PER, (core + 1) * M_PER)
        signneg = np.ones((B, M_PER), np.float32)
        for j in range(M_PER):
            signneg[core * M_PER + j, j] = -1.0
        pat = np.ones((NPAD, D), np.float32)
        pat[:NPF] = image_token_features[s].reshape(NPF, D)
        in_maps.append({
            "patches": pat.astype(BF16),
            "cT": cT,
            "cnat": cnat8,
            "GT": GT,
            "img": image_features[s].astype(BF16),
            "txt": txt_bf,
            "signneg": signneg,
            "ident": ident,
        })
    return in_maps, C, t, bias


def _run(inputs, trace=False, tmpdir=None):
    in_maps, C, t, bias = _prepare(inputs)
    key = (C, t, bias)
    if key not in _cache:
        _cache[key] = _build(C, t, bias)
    nc = _cache[key]
    kwargs = {}
    if trace:
        _install_trace_hook()
        kwargs = dict(trace=True, tmpdir=tmpdir)
    res = run_bass_kernel_spmd(nc, in_maps, core_ids=list(range(N_CORES)),
                               **kwargs)
    it_sum = sum(float(r["it_el"].astype(np.float64).sum()) for r in res.results)
    rc_sum = sum(float(r["rc_el"].astype(np.float64).sum()) for r in res.results)
    it_loss = it_sum / (B * B)
    rc_loss = rc_sum / (B * B)
    total = it_loss + 0.5 * rc_loss
    out = (np.float32(total), np.float32(it_loss), np.float32(rc_loss))
    return out, res


def kernel(**inputs):
    out, _ = _run(inputs)
    return out


# revision 8
# speedup vs baseline: 1.0232x; 1.0232x over previous
"""ConceptCLIP loss kernel for 8x Trainium2 NeuronCores (Bass/Tile).

Strategy (data-parallel over the image batch axis m):
  - Each core owns 16 of the 128 images; concept/text features are
    replicated. Concepts are host-packed (only w < counts[v] kept) and the
    concept L2 norm is deferred into the host-built gather matrix G.
  - The big patch x concept similarity matmul runs in fp8e4 with
    MatmulPerfMode.DoubleRow (K=256 per instruction, 2x bf16 = 157 TF/s;
    measured ~168 ns per 392-col matmul = ~1 col/cycle). fp8 error
    budget: cosine rms error ~3e-3 on values ~0.1 -> loss error ~5e-5
    rel, far inside the 2e-2 gate.
  - Patch prep (25 blocks of 128 flat patch rows, bf16 from host): ACT
    square+accum -> sqrt -> DVE recip gives per-patch 1/||p||; DVE scales
    straight to fp8. Blocks 0..12 (needed by main pass 0) transpose on
    the otherwise-idle PE in fp8 (hardware requires stride-2 output, so
    rhs is pair-packed [.., NPAD, 2] and the matmuls read the stride-2
    view); a DVE bitcast-to-bf16 copy moves pair-packed PSUM bytes to
    rhs at 2x rate, eliminating the ACT cast pass. Blocks 13..24 are
    prepped during pass 0 via XBAR dma_start_transpose (no PE/PSUM use).
  - Main loop: one pass per 8-image half; per 128-concept chunk c: 3
    DoubleRow k-pairs x 4 PSUM-bank chains of 392 cols (2 images each),
    then a single 4D DVE reduce_max (f32 PSUM -> bf16 maxcol).
  - Concept norms + G scaling and the IT softplus output are interleaved
    into pass 1; S = G_eff^T @ maxcol runs in bf16 after pass 1; softplus
    loss elements are computed on device and summed on host.
  - ACT tables (Square/Sqrt/Copy) preload during the DMA fill; Exp/Ln
    load during pass 1. Tiny constants ride the sync queue, patches +
    concepts the gpsimd queue, ordered so early prep blocks are not
    stuck behind bulk transfers on the shared DMA ring.
"""

import math
import os
import sys

for _p in ("/opt/trn_rl_repo", "/root/.axon_site/_ro/trn_rl_repo"):
    if os.path.isdir(_p) and _p not in sys.path:
        sys.path.insert(0, _p)

import ml_dtypes
import numpy as np

import concourse.tile as tile
from concourse import bacc, mybir
from concourse.bass_utils import run_bass_kernel_spmd

BF16 = ml_dtypes.bfloat16
FP8 = ml_dtypes.float8_e4m3

N_CORES = 8
B, NPATCH, D, W = 128, 196, 768, 32
M_PER = B // N_CORES          # 16 images per core
KC = D // 128                 # 6 contraction chunks
NPF = M_PER * NPATCH          # 3136 flat patch columns per core
NBLK = math.ceil(NPF / 128)   # 25 prep blocks (24x128 + 64)
NPAD = NBLK * 128             # 3200 padded patch rows
HALF = NPF // 2               # 1568 columns per main-loop pass
CHW = 2 * NPATCH              # 392-column chains (2 images per PSUM bank)

F32 = mybir.dt.float32
BF = mybir.dt.bfloat16
F8 = mybir.dt.float8e4
AX = mybir.AxisListType
AF = mybir.ActivationFunctionType
PM = mybir.MatmulPerfMode

_cache = {}


def _build(C, t, bias):
    """Build + compile the per-core Bass program. C = number of 128-row packed
    concept chunks; t/bias are compile-time scalar constants."""
    P = C * 128
    nc = bacc.Bacc("TRN2", target_bir_lowering=False, debug=False,
                   num_devices=N_CORES)

    d_patches = nc.dram_tensor("patches", (NPAD, D), BF, kind="ExternalInput")
    d_cT = nc.dram_tensor("cT", (KC, 128, P), F8, kind="ExternalInput")
    d_cnat = nc.dram_tensor("cnat", (P, D), F8, kind="ExternalInput")
    d_GT = nc.dram_tensor("GT", (C, 128, B), F32, kind="ExternalInput")
    d_img = nc.dram_tensor("img", (M_PER, D), BF, kind="ExternalInput")
    d_txt = nc.dram_tensor("txt", (B, D), BF, kind="ExternalInput")
    d_sign = nc.dram_tensor("signneg", (B, M_PER), F32, kind="ExternalInput")
    d_ident = nc.dram_tensor("ident", (128, 128), BF, kind="ExternalInput")
    d_rc = nc.dram_tensor("rc_el", (B, M_PER), F32, kind="ExternalOutput")
    d_it = nc.dram_tensor("it_el", (B, M_PER), F32, kind="ExternalOutput")

    with tile.TileContext(nc) as tc:
        with (
            tc.tile_pool(name="consts", bufs=1) as consts,
            tc.tile_pool(name="work", bufs=3) as work,
            tc.tile_pool(name="small", bufs=4) as small,
            tc.tile_pool(name="psum", bufs=2, space="PSUM") as psum,
        ):
            # preload the ACT tables needed during prep while DMAs fill
            warm = small.tile([1, 1], F32, tag="warm")
            nc.vector.memset(warm[:], 1.0)
            nc.scalar.activation(out=warm[:], in_=warm[:], func=AF.Square)
            nc.scalar.sqrt(warm[:], warm[:])
            nc.scalar.activation(out=warm[:], in_=warm[:], func=AF.Copy,
                                 bias=0.0, scale=1.0)

            # big SBUF residents
            rhs = consts.tile([128, KC, NPAD, 2], F8, tag="rhs")    # patchesT (stride-2)
            cT = consts.tile([128, KC, P], F8, tag="cT")            # conceptsT
            cnat = consts.tile([128, C, D], F8, tag="cnat")
            GT = consts.tile([128, C, B], F32, tag="GT")
            GTb = consts.tile([128, C, B], BF, tag="GTb")
            maxcol = consts.tile([128, C, M_PER], BF, tag="maxcol")
            rnorm = consts.tile([128, C], F32, tag="rnorm")
            txtT = consts.tile([128, KC, 128], BF, tag="txtT")
            imgT = consts.tile([128, KC, M_PER], BF, tag="imgT")
            yit = consts.tile([B, M_PER], F32, tag="yit")

            # ---------- loads: tiny consts on sync, patches first on gpsimd
            ident = consts.tile([128, 128], BF, tag="ident")
            nc.sync.dma_start(out=ident[:], in_=d_ident.ap())
            ident8 = consts.tile([128, 128], F8, tag="ident8")
            nc.scalar.copy(out=ident8[:], in_=ident[:])
            txt_t = work.tile([128, D], BF, tag="txtld", bufs=1)
            nc.sync.dma_start(out=txt_t[:], in_=d_txt.ap())
            img_t = work.tile([128, D], BF, tag="imgld", bufs=1)
            nc.sync.dma_start(out=img_t[0:M_PER], in_=d_img.ap())
            sign = consts.tile([B, M_PER], F32, tag="sign")
            nc.sync.dma_start(out=sign[:], in_=d_sign.ap())

            # patch loads + cT keep the DMA ring clear for early prep; the
            # cnat/GT loads are dispatched mid-pass-0 (see extras below).
            nats = []
            LOADB = 5  # blocks per load DMA
            for g in range(math.ceil(NBLK / LOADB)):
                b0, b1 = g * LOADB, min((g + 1) * LOADB, NBLK)
                natg = work.tile([128, LOADB, D], BF, tag="natg", bufs=6,
                                 name=f"natg{g}")
                src = d_patches.ap()[b0 * 128:b1 * 128, :].rearrange(
                    "(blk p) d -> p blk d", p=128)
                nc.gpsimd.dma_start(out=natg[:, 0:b1 - b0, :], in_=src)
                nats.append(natg)

            nc.gpsimd.dma_start(out=cT[:], in_=d_cT.ap().rearrange(
                "k p n -> p k n"))
            nc.gpsimd.dma_start(out=cnat[:], in_=d_cnat.ap().rearrange(
                "(c p) d -> p c d", p=128))
            nc.gpsimd.dma_start(out=GT[:], in_=d_GT.ap().rearrange(
                "c p v -> p c v"))

            def block_rinv_bn(nat, nrows):
                # per-patch 1/||row|| via DVE bn_stats (pre-main blocks)
                st = small.tile([128, 2, 6], F32, tag="st", bufs=4)
                nc.vector.bn_stats(out=st[:nrows, 0, :],
                                   in_=nat[:nrows, 0:D // 2])
                nc.vector.bn_stats(out=st[:nrows, 1, :],
                                   in_=nat[:nrows, D // 2:D])
                ag = small.tile([128, 2], F32, tag="ag", bufs=4)
                nc.vector.bn_aggr(out=ag[:nrows], in_=st[:nrows])
                m2 = small.tile([128, 2], F32, tag="m2", bufs=4)
                nc.scalar.activation(out=m2[:nrows, 0:1], in_=ag[:nrows, 0:1],
                                     func=AF.Square)
                nc.vector.tensor_add(m2[:nrows, 1:2], ag[:nrows, 1:2],
                                     m2[:nrows, 0:1])
                rinv = small.tile([128, 1], F32, tag="rinv", bufs=8)
                nc.scalar.activation(out=rinv[:nrows], in_=m2[:nrows, 1:2],
                                     func=AF.Sqrt, scale=float(D))
                nc.vector.reciprocal(rinv[:nrows], rinv[:nrows])
                return rinv

            def block_rinv_act(nat, nrows):
                ssq = small.tile([128, 1], F32, tag="ssq", bufs=8)
                scr = work.tile([128, D], BF, tag="scr", bufs=2)
                nc.scalar.activation(out=scr[:nrows], in_=nat[:nrows],
                                     func=AF.Square, accum_out=ssq[:nrows])
                nc.scalar.sqrt(ssq[:nrows], ssq[:nrows])
                rinv = small.tile([128, 1], F32, tag="rinv", bufs=8)
                nc.vector.reciprocal(rinv[:nrows], ssq[:nrows])
                return rinv

            def prep_block_xbar(b):
                # mid-main path: XBAR transpose, no PSUM / PE use
                nrows = min(128, NPF - b * 128)
                nat = nats[b // LOADB][:, b % LOADB, :]
                rinv = block_rinv_act(nat, nrows)
                nrm = work.tile([128, D], BF, tag="nrm", bufs=6)
                nc.vector.tensor_scalar_mul(nrm[:nrows], nat[:nrows],
                                            rinv[:nrows])
                tp = work.tile([128, KC, 128], BF, tag="tp", bufs=6)
                nc.sync.dma_start_transpose(out=tp[:, :, 0:nrows],
                                            in_=nrm[0:nrows, :])
                nc.scalar.copy(out=rhs[:, :, b * 128:b * 128 + nrows, 0],
                               in_=tp[:, :, 0:nrows])

            def prep_block_pe(b):
                # pre-main path: DVE scale straight to fp8 -> PE fp8
                # transpose (stride-2 PSUM out, a hardware requirement) ->
                # DVE bitcast copy moves the pair-packed bytes to rhs at
                # bf16 2x rate; no ACT cast pass at all.
                nrows = min(128, NPF - b * 128)
                nat = nats[b // LOADB][:, b % LOADB, :]
                rinv = block_rinv_act(nat, nrows)
                nrm8 = work.tile([128, D], F8, tag="nrm8", bufs=6)
                nc.vector.tensor_scalar_mul(nrm8[:nrows], nat[:nrows],
                                            rinv[:nrows])
                ps = psum.tile([128, KC, 128, 2], F8, tag="mm", name="ps_d")
                for k in range(KC):
                    nc.tensor.transpose(ps[:, k, 0:nrows, 0],
                                        nrm8[:nrows, k * 128:(k + 1) * 128],
                                        ident8[:nrows, :nrows])
                nc.vector.tensor_copy(
                    out=rhs[:, :, b * 128:b * 128 + nrows, :].bitcast(BF),
                    in_=ps[:, :, 0:nrows, :].bitcast(BF))

            def concept_chunk(c0, c1):
                # concept norms c0..c1 (deferred into GTb); interleaved into
                # the pass-0 emission so GTb is ready by pass-0 end
                for c in range(c0, min(c1, C)):
                    ssqc = small.tile([128, 1], F32, tag="ssq", bufs=8)
                    scrc = work.tile([128, D], F8, tag="scrc", bufs=2)
                    nc.scalar.activation(out=scrc[:], in_=cnat[:, c, :],
                                         func=AF.Square, accum_out=ssqc[:])
                    nc.scalar.sqrt(ssqc[:], ssqc[:])
                    nc.vector.reciprocal(rnorm[:, c:c + 1], ssqc[:])
                    nc.vector.tensor_scalar_mul(GTb[:, c, :], GT[:, c, :],
                                                rnorm[:, c:c + 1])

            def late_tables():
                nc.scalar.activation(out=warm[:], in_=warm[:], func=AF.Exp)
                nc.scalar.activation(out=warm[:], in_=warm[:], func=AF.Ln)

            # ---------- text / image CLS features (bf16 IT path) ----------
            def rownorm_recip(src_ap, nrows, rinv_ap):
                scr = work.tile([128, D], BF, tag="scrb", bufs=2)
                ssq = small.tile([128, 1], F32, tag="ssq", bufs=8)
                nc.scalar.activation(out=scr[:nrows], in_=src_ap,
                                     func=AF.Square, accum_out=ssq[:nrows])
                nc.scalar.sqrt(ssq[:nrows], ssq[:nrows])
                nc.vector.reciprocal(rinv_ap, ssq[:nrows])

            def norm_transpose(src_tile, nrows, dst, copy_eng):
                rinv = small.tile([128, 1], F32, tag="rinv", bufs=8)
                rownorm_recip(src_tile[:nrows], nrows, rinv[:nrows])
                nrm = work.tile([128, D], BF, tag="nrm", bufs=6)
                nc.vector.tensor_scalar_mul(nrm[:nrows], src_tile[:nrows],
                                            rinv[:nrows])
                ps = psum.tile([128, KC, 128], BF, tag="mm", name="ps_t")
                for k in range(KC):
                    nc.tensor.transpose(ps[:, k, 0:nrows],
                                        nrm[:nrows, k * 128:(k + 1) * 128],
                                        ident[:nrows, :nrows])
                copy_eng(out=dst[:, :, 0:nrows], in_=ps[:, :, 0:nrows])

            # ---------- upfront prep + IT path -----------------------------
            # pass 0 needs columns 0..1567 = blocks 0..12, prepped before the
            # main loop; sumsq/scale engines alternate to balance ACT vs DVE.
            for b in range(5):
                prep_block_pe(b)

            # txt/img/IT emitted early so their PE ops sit ahead of the main
            # matmuls in the queue but their ACT/DVE ops don't delay block 0.
            norm_transpose(txt_t, 128, txtT, nc.vector.tensor_copy)
            norm_transpose(img_t, M_PER, imgT, nc.scalar.copy)
            itps = psum.tile([128, 512], F32, tag="mm", name="itps")
            for k in range(KC):
                nc.tensor.matmul(itps[:, 0:M_PER], lhsT=txtT[:, k, :],
                                 rhs=imgT[:, k, :], start=(k == 0),
                                 stop=(k == KC - 1))
            nc.scalar.activation(out=yit[:], in_=itps[:, 0:M_PER],
                                 func=AF.Copy, bias=float(bias),
                                 scale=float(t))

            for b in range(5, 13):
                prep_block_pe(b)

            def softplus_out(y_ap, d_out):
                el = small.tile([B, M_PER], F32, tag="el", name="el")
                nc.scalar.activation(out=el[:], in_=y_ap, func=AF.Exp)
                nc.vector.tensor_scalar_add(el[:], el[:], 1.0)
                nc.scalar.activation(out=el[:], in_=el[:], func=AF.Ln)
                nc.sync.dma_start(out=d_out.ap(), in_=el[:])

            # ---------- main loop: fp8 DoubleRow patch x concept -----------
            def main_pass(pt, extras=()):
                extras = dict(extras)
                base = pt * HALF
                for c in range(C):
                    ps4 = psum.tile([128, 4, 512], F32, tag="mm", name="ps4")
                    for kp in range(KC // 2):
                        for i in range(4):
                            nc.tensor.matmul(
                                ps4[:, i, 0:CHW],
                                lhsT=cT[:, 2 * kp:2 * kp + 2,
                                        c * 128:(c + 1) * 128],
                                rhs=rhs[:, 2 * kp:2 * kp + 2,
                                        base + i * CHW:base + (i + 1) * CHW,
                                        0],
                                start=(kp == 0), stop=(kp == KC // 2 - 1),
                                perf_mode=PM.DoubleRow)
                    nc.vector.reduce_max(
                        out=maxcol[:, c, pt * 8:pt * 8 + 8].rearrange(
                            "p (i m) -> p i m", i=4),
                        in_=ps4[:, :, 0:CHW].rearrange(
                            "p i (m n) -> p i m n", m=2),
                        axis=AX.X)
                    if c in extras:
                        extras[c]()
                for k in sorted(extras):
                    if k >= C:  # safety: C is data-dependent
                        extras[k]()

            ex0 = {c: (lambda b=13 + c: prep_block_xbar(b))
                   for c in range(12)}
            ex1 = {0: lambda: concept_chunk(0, 6),
                   1: lambda: concept_chunk(6, 12),
                   2: lambda: concept_chunk(12, C),
                   3: late_tables,
                   4: lambda: (nc.vector.tensor_mul(yit[:], yit[:], sign[:]),
                               softplus_out(yit[:], d_it))}
            main_pass(0, ex0)
            main_pass(1, ex1)

            # ---------- S = G_eff^T @ maxcol (bf16), losses ----------------
            sps = psum.tile([128, 512], F32, tag="mm", name="sps")
            for c in range(C):
                nc.tensor.matmul(sps[:, 0:M_PER],
                                 lhsT=GTb[:, c, :],
                                 rhs=maxcol[:, c, :], start=(c == 0),
                                 stop=(c == C - 1))

            yrc = small.tile([B, M_PER], F32, tag="y")
            nc.scalar.activation(out=yrc[:], in_=sps[:, 0:M_PER],
                                 func=AF.Copy, bias=float(bias),
                                 scale=float(t))
            nc.vector.tensor_mul(yrc[:], yrc[:], sign[:])
            softplus_out(yrc[:], d_rc)

    nc.compile()
    return nc


def _install_trace_hook():
    """Register the axon NTFF profiling hook (missing from this image) so
    run_bass_kernel_spmd(trace=True) can capture HW exec time."""
    import contextlib
    import ctypes
    import types

    import concourse.bass_utils as bu

    if "antenv.axon_hooks" in sys.modules:
        return
    so_path = "/opt/axon/libaxon_pjrt.so"

    def _make_hook():
        lib = ctypes.CDLL(so_path)
        if not hasattr(lib, "axon_start_nrt_profile"):
            return None
        lib.axon_start_nrt_profile.argtypes = [ctypes.POINTER(ctypes.c_int64),
                                               ctypes.c_size_t]
        lib.axon_start_nrt_profile.restype = ctypes.c_int64
        lib.axon_stop_nrt_profile.argtypes = [ctypes.c_char_p]
        lib.axon_stop_nrt_profile.restype = ctypes.c_int64

        @contextlib.contextmanager
        def _hook(output_dir, device_ids):
            import jax
            jax.devices()
            if device_ids:
                ids = (ctypes.c_int64 * len(device_ids))(*device_ids)
                rc = lib.axon_start_nrt_profile(ids, len(device_ids))
            else:
                rc = lib.axon_start_nrt_profile(None, 0)
            if rc != 0:
                raise RuntimeError(f"axon_start_nrt_profile rc={rc}")
            try:
                yield
            finally:
                n = lib.axon_stop_nrt_profile(str(output_dir).encode())
                print(f"profile: {n} file(s) written to {output_dir}",
                      file=sys.stderr)

        return _hook

    mod = types.ModuleType("antenv.axon_hooks")
    mod.get_axon_ntff_profile_hook = _make_hook
    sys.modules["antenv.axon_hooks"] = mod
    bu.upload_artifacts = lambda tmpdir: tmpdir  # no S3 in this container


def _prepare(inputs):
    image_features = np.asarray(inputs["image_features"], np.float32)
    text_features = np.asarray(inputs["text_features"], np.float32)
    image_token_features = np.asarray(inputs["image_token_features"], np.float32)
    concept_text_features = np.asarray(inputs["concept_text_features"], np.float32)
    counts = np.asarray(inputs["concept_counts"]).astype(np.int64)
    t = float(np.exp(np.clip(np.float32(inputs["logit_scale"]), -10.0, 10.0)))
    bias = float(np.float32(inputs["logit_bias"]))

    # pack concepts: keep only w < counts[v]; pad rows with ones (zero weight)
    vidx = np.repeat(np.arange(B), counts)
    widx = np.concatenate([np.arange(c) for c in counts])
    P = len(vidx)
    C = math.ceil(P / 128)
    Ppad = C * 128
    cnat = np.ones((Ppad, D), np.float32)
    cnat[:P] = concept_text_features[vidx, widx]
    cnat8 = cnat.astype(FP8)
    cT = np.ascontiguousarray(cnat8.T).reshape(KC, 128, Ppad)

    G = np.zeros((Ppad, B), np.float32)
    G[np.arange(P), vidx] = 1.0 / counts[vidx]
    GT = G.reshape(C, 128, B)

    txt_bf = text_features.astype(BF16)
    ident = np.eye(128, dtype=BF16)

    in_maps = []
    for core in range(N_CORES):
        s = slice(core * M_PER, (core + 1) * M_PER)
        signneg = np.ones((B, M_PER), np.float32)
        for j in range(M_PER):
            signneg[core * M_PER + j, j] = -1.0
        pat = np.ones((NPAD, D), np.float32)
        pat[:NPF] = image_token_features[s].reshape(NPF, D)
        in_maps.append({
            "patches": pat.astype(BF16),
            "cT": cT,
            "cnat": cnat8,
            "GT": GT,
            "img": image_features[s].astype(BF16),
            "txt": txt_bf,
            "signneg": signneg,
            "ident": ident,
        })
    return in_maps, C, t, bias


def _run(inputs, trace=False, tmpdir=None):
    in_maps, C, t, bias = _prepare(inputs)
    key = (C, t, bias)
    if key not in _cache:
        _cache[key] = _build(C, t, bias)
    nc = _cache[key]
    kwargs = {}
    if trace:
        _install_trace_hook()
        kwargs = dict(trace=True, tmpdir=tmpdir)
    res = run_bass_kernel_spmd(nc, in_maps, core_ids=list(range(N_CORES)),
                               **kwargs)
    it_sum = sum(float(r["it_el"].astype(np.float64).sum()) for r in res.results)
    rc_sum = sum(float(r["rc_el"].astype(np.float64).sum()) for r in res.results)
    it_loss = it_sum / (B * B)
    rc_loss = rc_sum / (B * B)
    total = it_loss + 0.5 * rc_loss
    out = (np.float32(total), np.float32(it_loss), np.float32(rc_loss))
    return out, res


def kernel(**inputs):
    out, _ = _run(inputs)
    return out


# revision 9
# speedup vs baseline: 1.0426x; 1.0190x over previous
"""ConceptCLIP loss kernel for 8x Trainium2 NeuronCores (Bass/Tile), v2.

Strategy (data-parallel over the image batch axis m):
  - Each core owns 16 of the 128 images; concepts/text features are
    replicated. Concepts are host-packed (only w < counts[v] kept) and the
    concept L2 norm is deferred into the host-built gather matrix G.
  - The big patch x concept similarity matmul runs in fp8e4 with
    MatmulPerfMode.DoubleRow (K=256 per instruction, 2x bf16 throughput).
    fp8 error analysis: cosine rms error ~3e-3 against values +-0.15 ->
    rc_loss relative error ~0.2%, far inside the 2e-2 gate.
  - Patches are shipped fp8 (n,d) flat (3136 rows); per 128-row block:
    ACT square+accum -> sqrt -> DVE recip -> DVE scale to bf16, then an
    XBAR dma_start_transpose lands the normalized block directly in the
    (d-chunk, n) rhs layout (bf16 staging), cast-copied to fp8. The PE
    never runs patch transposes.
  - Main loop: one pass per 8-image half, per 128-concept chunk c: 3
    DoubleRow k-pairs x 4 PSUM-bank chains of 392 cols (2 images each),
    then a single 4D DVE reduce_max -> maxcol bf16.
  - S = G_eff^T @ maxcol in bf16, IT-align on CLS features in bf16,
    softplus losses on device, host sums the per-element losses.
"""

import math
import os
import sys

for _p in ("/opt/trn_rl_repo", "/root/.axon_site/_ro/trn_rl_repo"):
    if os.path.isdir(_p) and _p not in sys.path:
        sys.path.insert(0, _p)

import ml_dtypes
import numpy as np

import concourse.tile as tile
from concourse import bacc, mybir
from concourse.bass_utils import run_bass_kernel_spmd

BF16 = ml_dtypes.bfloat16
FP8 = ml_dtypes.float8_e4m3

N_CORES = 8
B, NPATCH, D, W = 128, 196, 768, 32
M_PER = B // N_CORES          # 16 images per core
KC = D // 128                 # 6 contraction chunks
NPF = M_PER * NPATCH          # 3136 flat patch columns per core
NBLK = math.ceil(NPF / 128)   # 25 prep blocks (24x128 + 64)
NPAD = NBLK * 128             # 3200 padded patch rows
HALF = NPF // 2               # 1568 columns per main-loop pass
CHW = 2 * NPATCH              # 392-column chains (2 images per PSUM bank)

F32 = mybir.dt.float32
BF = mybir.dt.bfloat16
F8 = mybir.dt.float8e4
AX = mybir.AxisListType
AF = mybir.ActivationFunctionType
PM = mybir.MatmulPerfMode

_cache = {}


def _build(C, t, bias):
    """Build + compile the per-core Bass program. C = number of 128-row packed
    concept chunks; t/bias are compile-time scalar constants."""
    P = C * 128
    nc = bacc.Bacc("TRN2", target_bir_lowering=False, debug=False,
                   num_devices=N_CORES)

    d_patches = nc.dram_tensor("patches", (NPAD, D), BF, kind="ExternalInput")
    d_cT = nc.dram_tensor("cT", (KC, 128, P), F8, kind="ExternalInput")
    d_cnat = nc.dram_tensor("cnat", (P, D), F8, kind="ExternalInput")
    d_GT = nc.dram_tensor("GT", (C, 128, B), F32, kind="ExternalInput")
    d_img = nc.dram_tensor("img", (M_PER, D), BF, kind="ExternalInput")
    d_txt = nc.dram_tensor("txt", (B, D), BF, kind="ExternalInput")
    d_sign = nc.dram_tensor("signneg", (B, M_PER), F32, kind="ExternalInput")
    d_ident = nc.dram_tensor("ident", (128, 128), BF, kind="ExternalInput")
    d_rc = nc.dram_tensor("rc_el", (B, M_PER), F32, kind="ExternalOutput")
    d_it = nc.dram_tensor("it_el", (B, M_PER), F32, kind="ExternalOutput")

    with tile.TileContext(nc) as tc:
        with (
            tc.tile_pool(name="consts", bufs=1) as consts,
            tc.tile_pool(name="work", bufs=3) as work,
            tc.tile_pool(name="small", bufs=4) as small,
            tc.tile_pool(name="psum", bufs=2, space="PSUM") as psum,
        ):
            # preload the ACT tables needed during prep while DMAs fill
            warm = small.tile([1, 1], F32, tag="warm")
            nc.vector.memset(warm[:], 1.0)
            nc.scalar.activation(out=warm[:], in_=warm[:], func=AF.Square)
            nc.scalar.sqrt(warm[:], warm[:])
            nc.scalar.activation(out=warm[:], in_=warm[:], func=AF.Copy,
                                 bias=0.0, scale=1.0)

            # big SBUF residents
            rhs = consts.tile([128, KC, NPAD, 2], F8, tag="rhs")    # patchesT (stride-2)
            cT = consts.tile([128, KC, P], F8, tag="cT")            # conceptsT
            cnat = consts.tile([128, C, D], F8, tag="cnat")
            GT = consts.tile([128, C, B], F32, tag="GT")
            GTb = consts.tile([128, C, B], BF, tag="GTb")
            maxcol = consts.tile([128, C, M_PER], BF, tag="maxcol")
            rnorm = consts.tile([128, C], F32, tag="rnorm")
            txtT = consts.tile([128, KC, 128], BF, tag="txtT")
            imgT = consts.tile([128, KC, M_PER], BF, tag="imgT")
            yit = consts.tile([B, M_PER], F32, tag="yit")

            # ---------- loads: tiny consts on sync, patches first on gpsimd
            ident = consts.tile([128, 128], BF, tag="ident")
            nc.sync.dma_start(out=ident[:], in_=d_ident.ap())
            ident8 = consts.tile([128, 128], F8, tag="ident8")
            nc.scalar.copy(out=ident8[:], in_=ident[:])
            txt_t = work.tile([128, D], BF, tag="txtld", bufs=1)
            nc.sync.dma_start(out=txt_t[:], in_=d_txt.ap())
            img_t = work.tile([128, D], BF, tag="imgld", bufs=1)
            nc.sync.dma_start(out=img_t[0:M_PER], in_=d_img.ap())
            sign = consts.tile([B, M_PER], F32, tag="sign")
            nc.sync.dma_start(out=sign[:], in_=d_sign.ap())

            # patch loads + cT keep the DMA ring clear for early prep; the
            # cnat/GT loads are dispatched mid-pass-0 (see extras below).
            nats = []
            LOADB = 5  # blocks per load DMA
            for g in range(math.ceil(NBLK / LOADB)):
                b0, b1 = g * LOADB, min((g + 1) * LOADB, NBLK)
                natg = work.tile([128, LOADB, D], BF, tag="natg", bufs=6,
                                 name=f"natg{g}")
                src = d_patches.ap()[b0 * 128:b1 * 128, :].rearrange(
                    "(blk p) d -> p blk d", p=128)
                nc.gpsimd.dma_start(out=natg[:, 0:b1 - b0, :], in_=src)
                nats.append(natg)

            nc.gpsimd.dma_start(out=cT[:], in_=d_cT.ap().rearrange(
                "k p n -> p k n"))
            nc.gpsimd.dma_start(out=cnat[:], in_=d_cnat.ap().rearrange(
                "(c p) d -> p c d", p=128))
            nc.gpsimd.dma_start(out=GT[:], in_=d_GT.ap().rearrange(
                "c p v -> p c v"))

            def block_rinv_bn(nat, nrows):
                # per-patch 1/||row|| via DVE bn_stats (pre-main blocks)
                st = small.tile([128, 2, 6], F32, tag="st", bufs=4)
                nc.vector.bn_stats(out=st[:nrows, 0, :],
                                   in_=nat[:nrows, 0:D // 2])
                nc.vector.bn_stats(out=st[:nrows, 1, :],
                                   in_=nat[:nrows, D // 2:D])
                ag = small.tile([128, 2], F32, tag="ag", bufs=4)
                nc.vector.bn_aggr(out=ag[:nrows], in_=st[:nrows])
                m2 = small.tile([128, 2], F32, tag="m2", bufs=4)
                nc.scalar.activation(out=m2[:nrows, 0:1], in_=ag[:nrows, 0:1],
                                     func=AF.Square)
                nc.vector.tensor_add(m2[:nrows, 1:2], ag[:nrows, 1:2],
                                     m2[:nrows, 0:1])
                rinv = small.tile([128, 1], F32, tag="rinv", bufs=8)
                nc.scalar.activation(out=rinv[:nrows], in_=m2[:nrows, 1:2],
                                     func=AF.Sqrt, scale=float(D))
                nc.vector.reciprocal(rinv[:nrows], rinv[:nrows])
                return rinv

            def block_rinv_act(nat, nrows):
                ssq = small.tile([128, 1], F32, tag="ssq", bufs=8)
                scr = work.tile([128, D], BF, tag="scr", bufs=2)
                nc.scalar.activation(out=scr[:nrows], in_=nat[:nrows],
                                     func=AF.Square, accum_out=ssq[:nrows])
                nc.scalar.sqrt(ssq[:nrows], ssq[:nrows])
                rinv = small.tile([128, 1], F32, tag="rinv", bufs=8)
                nc.vector.reciprocal(rinv[:nrows], ssq[:nrows])
                return rinv

            def prep_block_xbar(b):
                # mid-main path: XBAR transpose, no PSUM / PE use
                nrows = min(128, NPF - b * 128)
                nat = nats[b // LOADB][:, b % LOADB, :]
                rinv = block_rinv_act(nat, nrows)
                nrm = work.tile([128, D], BF, tag="nrm", bufs=6)
                nc.vector.tensor_scalar_mul(nrm[:nrows], nat[:nrows],
                                            rinv[:nrows])
                tp = work.tile([128, KC, 128], BF, tag="tp", bufs=6)
                nc.sync.dma_start_transpose(out=tp[:, :, 0:nrows],
                                            in_=nrm[0:nrows, :])
                nc.scalar.copy(out=rhs[:, :, b * 128:b * 128 + nrows, 0],
                               in_=tp[:, :, 0:nrows])

            def prep_block_pe(b):
                # pre-main path: DVE scale straight to fp8 -> PE fp8
                # transpose (stride-2 PSUM out, a hardware requirement) ->
                # DVE bitcast copy moves the pair-packed bytes to rhs at
                # bf16 2x rate; no ACT cast pass at all.
                nrows = min(128, NPF - b * 128)
                nat = nats[b // LOADB][:, b % LOADB, :]
                rinv = block_rinv_act(nat, nrows)
                nrm8 = work.tile([128, D], F8, tag="nrm8", bufs=6)
                nc.vector.tensor_scalar_mul(nrm8[:nrows], nat[:nrows],
                                            rinv[:nrows])
                ps = psum.tile([128, KC, 128, 2], F8, tag="mm", name="ps_d")
                for k in range(KC):
                    nc.tensor.transpose(ps[:, k, 0:nrows, 0],
                                        nrm8[:nrows, k * 128:(k + 1) * 128],
                                        ident8[:nrows, :nrows])
                nc.vector.tensor_copy(
                    out=rhs[:, :, b * 128:b * 128 + nrows, :].bitcast(BF),
                    in_=ps[:, :, 0:nrows, :].bitcast(BF))

            def concept_chunk(c0, c1):
                # concept norms c0..c1 (deferred into GTb); interleaved into
                # the pass-0 emission so GTb is ready by pass-0 end
                for c in range(c0, min(c1, C)):
                    ssqc = small.tile([128, 1], F32, tag="ssq", bufs=8)
                    scrc = work.tile([128, D], F8, tag="scrc", bufs=2)
                    nc.scalar.activation(out=scrc[:], in_=cnat[:, c, :],
                                         func=AF.Square, accum_out=ssqc[:])
                    nc.scalar.sqrt(ssqc[:], ssqc[:])
                    nc.vector.reciprocal(rnorm[:, c:c + 1], ssqc[:])
                    nc.vector.tensor_scalar_mul(GTb[:, c, :], GT[:, c, :],
                                                rnorm[:, c:c + 1])

            def late_tables():
                nc.scalar.activation(out=warm[:], in_=warm[:], func=AF.Exp)
                nc.scalar.activation(out=warm[:], in_=warm[:], func=AF.Ln)

            # ---------- text / image CLS features (bf16 IT path) ----------
            def rownorm_recip(src_ap, nrows, rinv_ap):
                scr = work.tile([128, D], BF, tag="scrb", bufs=2)
                ssq = small.tile([128, 1], F32, tag="ssq", bufs=8)
                nc.scalar.activation(out=scr[:nrows], in_=src_ap,
                                     func=AF.Square, accum_out=ssq[:nrows])
                nc.scalar.sqrt(ssq[:nrows], ssq[:nrows])
                nc.vector.reciprocal(rinv_ap, ssq[:nrows])

            def norm_transpose(src_tile, nrows, dst, copy_eng):
                rinv = small.tile([128, 1], F32, tag="rinv", bufs=8)
                rownorm_recip(src_tile[:nrows], nrows, rinv[:nrows])
                nrm = work.tile([128, D], BF, tag="nrm", bufs=6)
                nc.vector.tensor_scalar_mul(nrm[:nrows], src_tile[:nrows],
                                            rinv[:nrows])
                ps = psum.tile([128, KC, 128], BF, tag="mm", name="ps_t")
                for k in range(KC):
                    nc.tensor.transpose(ps[:, k, 0:nrows],
                                        nrm[:nrows, k * 128:(k + 1) * 128],
                                        ident[:nrows, :nrows])
                copy_eng(out=dst[:, :, 0:nrows], in_=ps[:, :, 0:nrows])

            # ---------- upfront prep + IT path -----------------------------
            # pass 0 needs columns 0..1567 = blocks 0..12, prepped before the
            # main loop; sumsq/scale engines alternate to balance ACT vs DVE.
            for b in range(5):
                prep_block_pe(b)

            # txt/img/IT emitted early so their PE ops sit ahead of the main
            # matmuls in the queue but their ACT/DVE ops don't delay block 0.
            norm_transpose(txt_t, 128, txtT, nc.vector.tensor_copy)
            norm_transpose(img_t, M_PER, imgT, nc.scalar.copy)
            itps = psum.tile([128, 512], F32, tag="mm", name="itps")
            for k in range(KC):
                nc.tensor.matmul(itps[:, 0:M_PER], lhsT=txtT[:, k, :],
                                 rhs=imgT[:, k, :], start=(k == 0),
                                 stop=(k == KC - 1))
            nc.scalar.activation(out=yit[:], in_=itps[:, 0:M_PER],
                                 func=AF.Copy, bias=float(bias),
                                 scale=float(t))

            for b in range(5, 13):
                prep_block_pe(b)

            def softplus_out(y_ap, d_out):
                el = small.tile([B, M_PER], F32, tag="el", name="el")
                nc.scalar.activation(out=el[:], in_=y_ap, func=AF.Exp)
                nc.vector.tensor_scalar_add(el[:], el[:], 1.0)
                nc.scalar.activation(out=el[:], in_=el[:], func=AF.Ln)
                nc.sync.dma_start(out=d_out.ap(), in_=el[:])

            # ---------- main loop: fp8 DoubleRow patch x concept -----------
            def main_pass(pt, extras=()):
                extras = dict(extras)
                base = pt * HALF
                for c in range(C):
                    ps4 = psum.tile([128, 4, 512], F32, tag="mm", name="ps4")
                    for kp in range(KC // 2):
                        for i in range(4):
                            nc.tensor.matmul(
                                ps4[:, i, 0:CHW],
                                lhsT=cT[:, 2 * kp:2 * kp + 2,
                                        c * 128:(c + 1) * 128],
                                rhs=rhs[:, 2 * kp:2 * kp + 2,
                                        base + i * CHW:base + (i + 1) * CHW,
                                        0],
                                start=(kp == 0), stop=(kp == KC // 2 - 1),
                                perf_mode=PM.DoubleRow)
                    nc.vector.reduce_max(
                        out=maxcol[:, c, pt * 8:pt * 8 + 8].rearrange(
                            "p (i m) -> p i m", i=4),
                        in_=ps4[:, :, 0:CHW].rearrange(
                            "p i (m n) -> p i m n", m=2),
                        axis=AX.X)
                    if c in extras:
                        extras[c]()
                for k in sorted(extras):
                    if k >= C:  # safety: C is data-dependent
                        extras[k]()

            ex0 = {c: (lambda b=13 + c: prep_block_xbar(b))
                   for c in range(12)}
            ex1 = {0: lambda: concept_chunk(0, 6),
                   1: lambda: concept_chunk(6, 12),
                   2: lambda: concept_chunk(12, C),
                   3: late_tables,
                   4: lambda: (nc.vector.tensor_mul(yit[:], yit[:], sign[:]),
                               softplus_out(yit[:], d_it))}
            main_pass(0, ex0)
            main_pass(1, ex1)

            # ---------- S = G_eff^T @ maxcol (bf16), losses ----------------
            sps = psum.tile([128, 512], F32, tag="mm", name="sps")
            for c in range(C):
                nc.tensor.matmul(sps[:, 0:M_PER],
                                 lhsT=GTb[:, c, :],
                                 rhs=maxcol[:, c, :], start=(c == 0),
                                 stop=(c == C - 1))

            yrc = small.tile([B, M_PER], F32, tag="y")
            nc.scalar.activation(out=yrc[:], in_=sps[:, 0:M_PER],
                                 func=AF.Copy, bias=float(bias),
                                 scale=float(t))
            nc.vector.tensor_mul(yrc[:], yrc[:], sign[:])
            softplus_out(yrc[:], d_rc)

    nc.compile()
    return nc


def _install_trace_hook():
    """Register the axon NTFF profiling hook (missing from this image) so
    run_bass_kernel_spmd(trace=True) can capture HW exec time."""
    import contextlib
    import ctypes
    import types

    import concourse.bass_utils as bu

    if "antenv.axon_hooks" in sys.modules:
        return
    so_path = "/opt/axon/libaxon_pjrt.so"

    def _make_hook():
        lib = ctypes.CDLL(so_path)
        if not hasattr(lib, "axon_start_nrt_profile"):
            return None
        lib.axon_start_nrt_profile.argtypes = [ctypes.POINTER(ctypes.c_int64),
                                               ctypes.c_size_t]
        lib.axon_start_nrt_profile.restype = ctypes.c_int64
        lib.axon_stop_nrt_profile.argtypes = [ctypes.c_char_p]
        lib.axon_stop_nrt_profile.restype = ctypes.c_int64

        @contextlib.contextmanager
        def _hook(output_dir, device_ids):
            import jax
            jax.devices()
            if device_ids:
                ids = (ctypes.c_int64 * len(device_ids))(*device_ids)
                rc = lib.axon_start_nrt_profile(ids, len(device_ids))
            else:
                rc = lib.axon_start_nrt_profile(None, 0)
            if rc != 0:
                raise RuntimeError(f"axon_start_nrt_profile rc={rc}")
            try:
                yield
            finally:
                n = lib.axon_stop_nrt_profile(str(output_dir).encode())
                print(f"profile: {n} file(s) written to {output_dir}",
                      file=sys.stderr)

        return _hook

    mod = types.ModuleType("antenv.axon_hooks")
    mod.get_axon_ntff_profile_hook = _make_hook
    sys.modules["antenv.axon_hooks"] = mod
    bu.upload_artifacts = lambda tmpdir: tmpdir  # no S3 in this container


def _prepare(inputs):
    image_features = np.asarray(inputs["image_features"], np.float32)
    text_features = np.asarray(inputs["text_features"], np.float32)
    image_token_features = np.asarray(inputs["image_token_features"], np.float32)
    concept_text_features = np.asarray(inputs["concept_text_features"], np.float32)
    counts = np.asarray(inputs["concept_counts"]).astype(np.int64)
    t = float(np.exp(np.clip(np.float32(inputs["logit_scale"]), -10.0, 10.0)))
    bias = float(np.float32(inputs["logit_bias"]))

    # pack concepts: keep only w < counts[v]; pad rows with ones (zero weight)
    vidx = np.repeat(np.arange(B), counts)
    widx = np.concatenate([np.arange(c) for c in counts])
    P = len(vidx)
    C = math.ceil(P / 128)
    Ppad = C * 128
    cnat = np.ones((Ppad, D), np.float32)
    cnat[:P] = concept_text_features[vidx, widx]
    cnat8 = cnat.astype(FP8)
    cT = np.ascontiguousarray(cnat8.T).reshape(KC, 128, Ppad)

    G = np.zeros((Ppad, B), np.float32)
    G[np.arange(P), vidx] = 1.0 / counts[vidx]
    GT = G.reshape(C, 128, B)

    txt_bf = text_features.astype(BF16)
    ident = np.eye(128, dtype=BF16)

    in_maps = []
    for core in range(N_CORES):
        s = slice(core * M_PER, (core + 1) * M_PER)
        signneg = np.ones((B, M_PER), np.float32)
        for j in range(M_PER):
            signneg[core * M_PER + j, j] = -1.0
        pat = np.ones((NPAD, D), np.float32)
        pat[:NPF] = image_token_features[s].reshape(NPF, D)
        in_maps.append({
            "patches": pat.astype(BF16),
            "cT": cT,
            "cnat": cnat8,
            "GT": GT,
            "img": image_features[s].astype(BF16),
            "txt": txt_bf,
            "signneg": signneg,
            "ident": ident,
        })
    return in_maps, C, t, bias


def _run(inputs, trace=False, tmpdir=None):
    in_maps, C, t, bias = _prepare(inputs)
    key = (C, t, bias)
    if key not in _cache:
        _cache[key] = _build(C, t, bias)
    nc = _cache[key]
    kwargs = {}
    if trace:
        _install_trace_hook()
        kwargs = dict(trace=True, tmpdir=tmpdir)
    res = run_bass_kernel_spmd(nc, in_maps, core_ids=list(range(N_CORES)),
                               **kwargs)
    it_sum = sum(float(r["it_el"].astype(np.float64).sum()) for r in res.results)
    rc_sum = sum(float(r["rc_el"].astype(np.float64).sum()) for r in res.results)
    it_loss = it_sum / (B * B)
    rc_loss = rc_sum / (B * B)
    total = it_loss + 0.5 * rc_loss
    out = (np.float32(total), np.float32(it_loss), np.float32(rc_loss))
    return out, res


def kernel(**inputs):
    out, _ = _run(inputs)
    return out


# revision 10
# speedup vs baseline: 1.0510x; 1.0080x over previous
"""ConceptCLIP loss kernel for 8x Trainium2 NeuronCores (Bass/Tile), v2.

Strategy (data-parallel over the image batch axis m):
  - Each core owns 16 of the 128 images; concepts/text features are
    replicated. Concepts are host-packed (only w < counts[v] kept) and the
    concept L2 norm is deferred into the host-built gather matrix G.
  - The big patch x concept similarity matmul runs in fp8e4 with
    MatmulPerfMode.DoubleRow (K=256 per instruction, 2x bf16 throughput).
    fp8 error analysis: cosine rms error ~3e-3 against values +-0.15 ->
    rc_loss relative error ~0.2%, far inside the 2e-2 gate.
  - Patches are shipped fp8 (n,d) flat (3136 rows); per 128-row block:
    ACT square+accum -> sqrt -> DVE recip -> DVE scale to bf16, then an
    XBAR dma_start_transpose lands the normalized block directly in the
    (d-chunk, n) rhs layout (bf16 staging), cast-copied to fp8. The PE
    never runs patch transposes.
  - Main loop: one pass per 8-image half, per 128-concept chunk c: 3
    DoubleRow k-pairs x 4 PSUM-bank chains of 392 cols (2 images each),
    then a single 4D DVE reduce_max -> maxcol bf16.
  - S = G_eff^T @ maxcol in bf16, IT-align on CLS features in bf16,
    softplus losses on device, host sums the per-element losses.
"""

import math
import os
import sys

for _p in ("/opt/trn_rl_repo", "/root/.axon_site/_ro/trn_rl_repo"):
    if os.path.isdir(_p) and _p not in sys.path:
        sys.path.insert(0, _p)

import ml_dtypes
import numpy as np

import concourse.tile as tile
from concourse import bacc, mybir
from concourse.bass_utils import run_bass_kernel_spmd

BF16 = ml_dtypes.bfloat16
FP8 = ml_dtypes.float8_e4m3

N_CORES = 8
B, NPATCH, D, W = 128, 196, 768, 32
M_PER = B // N_CORES          # 16 images per core
KC = D // 128                 # 6 contraction chunks
NPF = M_PER * NPATCH          # 3136 flat patch columns per core
NBLK = math.ceil(NPF / 128)   # 25 prep blocks (24x128 + 64)
NPAD = NBLK * 128             # 3200 padded patch rows
HALF = NPF // 2               # 1568 columns per main-loop pass
CHW = 2 * NPATCH              # 392-column chains (2 images per PSUM bank)

F32 = mybir.dt.float32
BF = mybir.dt.bfloat16
F8 = mybir.dt.float8e4
AX = mybir.AxisListType
AF = mybir.ActivationFunctionType
PM = mybir.MatmulPerfMode

_cache = {}


def _build(C, t, bias):
    """Build + compile the per-core Bass program. C = number of 128-row packed
    concept chunks; t/bias are compile-time scalar constants."""
    P = C * 128
    nc = bacc.Bacc("TRN2", target_bir_lowering=False, debug=False,
                   num_devices=N_CORES)

    d_patches = nc.dram_tensor("patches", (NPAD, D), BF, kind="ExternalInput")
    d_cT = nc.dram_tensor("cT", (KC, 128, P), F8, kind="ExternalInput")
    d_cnat = nc.dram_tensor("cnat", (P, D), F8, kind="ExternalInput")
    d_GT = nc.dram_tensor("GT", (C, 128, B), F32, kind="ExternalInput")
    d_img = nc.dram_tensor("img", (M_PER, D), BF, kind="ExternalInput")
    d_txt = nc.dram_tensor("txt", (B, D), BF, kind="ExternalInput")
    d_sign = nc.dram_tensor("signneg", (B, M_PER), F32, kind="ExternalInput")
    d_ident = nc.dram_tensor("ident", (128, 128), BF, kind="ExternalInput")
    d_rc = nc.dram_tensor("rc_el", (B, M_PER), F32, kind="ExternalOutput")
    d_it = nc.dram_tensor("it_el", (B, M_PER), F32, kind="ExternalOutput")

    with tile.TileContext(nc) as tc:
        with (
            tc.tile_pool(name="consts", bufs=1) as consts,
            tc.tile_pool(name="work", bufs=3) as work,
            tc.tile_pool(name="small", bufs=4) as small,
            tc.tile_pool(name="psum", bufs=2, space="PSUM") as psum,
        ):
            # preload the ACT tables needed during prep while DMAs fill
            warm = small.tile([1, 1], F32, tag="warm")
            nc.vector.memset(warm[:], 1.0)
            nc.scalar.activation(out=warm[:], in_=warm[:], func=AF.Square)
            nc.scalar.sqrt(warm[:], warm[:])
            nc.scalar.activation(out=warm[:], in_=warm[:], func=AF.Copy,
                                 bias=0.0, scale=1.0)

            # big SBUF residents
            rhs = consts.tile([128, KC, NPAD, 2], F8, tag="rhs")    # patchesT (stride-2)
            cT = consts.tile([128, KC, P], F8, tag="cT")            # conceptsT
            cnat = consts.tile([128, C, D], F8, tag="cnat")
            GT = consts.tile([128, C, B], F32, tag="GT")
            GTb = consts.tile([128, C, B], BF, tag="GTb")
            maxcol = consts.tile([128, C, M_PER], BF, tag="maxcol")
            rnorm = consts.tile([128, C], F32, tag="rnorm")
            txtT = consts.tile([128, KC, 128], BF, tag="txtT")
            imgT = consts.tile([128, KC, M_PER], BF, tag="imgT")
            yit = consts.tile([B, M_PER], F32, tag="yit")

            # ---------- loads: tiny consts on sync, patches first on gpsimd
            ident = consts.tile([128, 128], BF, tag="ident")
            nc.sync.dma_start(out=ident[:], in_=d_ident.ap())
            ident8 = consts.tile([128, 128], F8, tag="ident8")
            nc.scalar.copy(out=ident8[:], in_=ident[:])
            txt_t = work.tile([128, D], BF, tag="txtld", bufs=1)
            nc.sync.dma_start(out=txt_t[:], in_=d_txt.ap())
            img_t = work.tile([128, D], BF, tag="imgld", bufs=1)
            nc.sync.dma_start(out=img_t[0:M_PER], in_=d_img.ap())
            sign = consts.tile([B, M_PER], F32, tag="sign")
            nc.sync.dma_start(out=sign[:], in_=d_sign.ap())

            # patch loads + cT keep the DMA ring clear for early prep; the
            # cnat/GT loads are dispatched mid-pass-0 (see extras below).
            nats = []
            LOADB = 5  # blocks per load DMA
            for g in range(math.ceil(NBLK / LOADB)):
                b0, b1 = g * LOADB, min((g + 1) * LOADB, NBLK)
                natg = work.tile([128, LOADB, D], BF, tag="natg", bufs=6,
                                 name=f"natg{g}")
                src = d_patches.ap()[b0 * 128:b1 * 128, :].rearrange(
                    "(blk p) d -> p blk d", p=128)
                nc.gpsimd.dma_start(out=natg[:, 0:b1 - b0, :], in_=src)
                nats.append(natg)

            nc.gpsimd.dma_start(out=cT[:], in_=d_cT.ap().rearrange(
                "k p n -> p k n"))
            nc.gpsimd.dma_start(out=cnat[:], in_=d_cnat.ap().rearrange(
                "(c p) d -> p c d", p=128))
            nc.gpsimd.dma_start(out=GT[:], in_=d_GT.ap().rearrange(
                "c p v -> p c v"))

            def block_rinv_bn(nat, nrows):
                # per-patch 1/||row|| via DVE bn_stats (pre-main blocks)
                st = small.tile([128, 2, 6], F32, tag="st", bufs=4)
                nc.vector.bn_stats(out=st[:nrows, 0, :],
                                   in_=nat[:nrows, 0:D // 2])
                nc.vector.bn_stats(out=st[:nrows, 1, :],
                                   in_=nat[:nrows, D // 2:D])
                ag = small.tile([128, 2], F32, tag="ag", bufs=4)
                nc.vector.bn_aggr(out=ag[:nrows], in_=st[:nrows])
                m2 = small.tile([128, 2], F32, tag="m2", bufs=4)
                nc.scalar.activation(out=m2[:nrows, 0:1], in_=ag[:nrows, 0:1],
                                     func=AF.Square)
                nc.vector.tensor_add(m2[:nrows, 1:2], ag[:nrows, 1:2],
                                     m2[:nrows, 0:1])
                rinv = small.tile([128, 1], F32, tag="rinv", bufs=8)
                nc.scalar.activation(out=rinv[:nrows], in_=m2[:nrows, 1:2],
                                     func=AF.Sqrt, scale=float(D))
                nc.vector.reciprocal(rinv[:nrows], rinv[:nrows])
                return rinv

            def block_rinv_act(nat, nrows):
                ssq = small.tile([128, 1], F32, tag="ssq", bufs=8)
                scr = work.tile([128, D], BF, tag="scr", bufs=2)
                nc.scalar.activation(out=scr[:nrows], in_=nat[:nrows],
                                     func=AF.Square, accum_out=ssq[:nrows])
                nc.scalar.sqrt(ssq[:nrows], ssq[:nrows])
                rinv = small.tile([128, 1], F32, tag="rinv", bufs=8)
                nc.vector.reciprocal(rinv[:nrows], ssq[:nrows])
                return rinv

            def prep_block_xbar(b):
                # mid-main path: XBAR transpose, no PSUM / PE use
                nrows = min(128, NPF - b * 128)
                nat = nats[b // LOADB][:, b % LOADB, :]
                rinv = block_rinv_act(nat, nrows)
                nrm = work.tile([128, D], BF, tag="nrm", bufs=6)
                nc.vector.tensor_scalar_mul(nrm[:nrows], nat[:nrows],
                                            rinv[:nrows])
                tp = work.tile([128, KC, 128], BF, tag="tp", bufs=6)
                nc.sync.dma_start_transpose(out=tp[:, :, 0:nrows],
                                            in_=nrm[0:nrows, :])
                nc.scalar.copy(out=rhs[:, :, b * 128:b * 128 + nrows, 0],
                               in_=tp[:, :, 0:nrows])

            def prep_block_pe(b):
                # pre-main path: DVE scale straight to fp8 -> PE fp8
                # transpose (stride-2 PSUM out, a hardware requirement) ->
                # DVE bitcast copy moves the pair-packed bytes to rhs at
                # bf16 2x rate; no ACT cast pass at all.
                nrows = min(128, NPF - b * 128)
                nat = nats[b // LOADB][:, b % LOADB, :]
                rinv = block_rinv_act(nat, nrows)
                nrm8 = work.tile([128, D], F8, tag="nrm8", bufs=6)
                nc.vector.tensor_scalar_mul(nrm8[:nrows], nat[:nrows],
                                            rinv[:nrows])
                ps = psum.tile([128, KC, 128, 2], F8, tag="mm", name="ps_d")
                for k in range(KC):
                    nc.tensor.transpose(ps[:, k, 0:nrows, 0],
                                        nrm8[:nrows, k * 128:(k + 1) * 128],
                                        ident8[:nrows, :nrows])
                nc.vector.tensor_copy(
                    out=rhs[:, :, b * 128:b * 128 + nrows, :].bitcast(BF),
                    in_=ps[:, :, 0:nrows, :].bitcast(BF))

            def concept_chunk(c0, c1):
                # concept norms c0..c1 (deferred into GTb); interleaved into
                # the pass-0 emission so GTb is ready by pass-0 end
                for c in range(c0, min(c1, C)):
                    ssqc = small.tile([128, 1], F32, tag="ssq", bufs=8)
                    scrc = work.tile([128, D], F8, tag="scrc", bufs=2)
                    nc.scalar.activation(out=scrc[:], in_=cnat[:, c, :],
                                         func=AF.Square, accum_out=ssqc[:])
                    nc.scalar.sqrt(ssqc[:], ssqc[:])
                    nc.vector.reciprocal(rnorm[:, c:c + 1], ssqc[:])
                    nc.vector.tensor_scalar_mul(GTb[:, c, :], GT[:, c, :],
                                                rnorm[:, c:c + 1])

            def late_tables():
                nc.scalar.activation(out=warm[:], in_=warm[:], func=AF.Exp)
                nc.scalar.activation(out=warm[:], in_=warm[:], func=AF.Ln)

            # ---------- text / image CLS features (bf16 IT path) ----------
            def rownorm_recip(src_ap, nrows, rinv_ap):
                scr = work.tile([128, D], BF, tag="scrb", bufs=2)
                ssq = small.tile([128, 1], F32, tag="ssq", bufs=8)
                nc.scalar.activation(out=scr[:nrows], in_=src_ap,
                                     func=AF.Square, accum_out=ssq[:nrows])
                nc.scalar.sqrt(ssq[:nrows], ssq[:nrows])
                nc.vector.reciprocal(rinv_ap, ssq[:nrows])

            def norm_transpose(src_tile, nrows, dst, copy_eng):
                rinv = small.tile([128, 1], F32, tag="rinv", bufs=8)
                rownorm_recip(src_tile[:nrows], nrows, rinv[:nrows])
                nrm = work.tile([128, D], BF, tag="nrm", bufs=6)
                nc.vector.tensor_scalar_mul(nrm[:nrows], src_tile[:nrows],
                                            rinv[:nrows])
                ps = psum.tile([128, KC, 128], BF, tag="mm", name="ps_t")
                for k in range(KC):
                    nc.tensor.transpose(ps[:, k, 0:nrows],
                                        nrm[:nrows, k * 128:(k + 1) * 128],
                                        ident[:nrows, :nrows])
                copy_eng(out=dst[:, :, 0:nrows], in_=ps[:, :, 0:nrows])

            # ---------- upfront prep + IT path -----------------------------
            # pass 0 needs columns 0..1567 = blocks 0..12, prepped before the
            # main loop; sumsq/scale engines alternate to balance ACT vs DVE.
            for b in range(5):
                prep_block_pe(b)

            # txt/img/IT emitted early so their PE ops sit ahead of the main
            # matmuls in the queue but their ACT/DVE ops don't delay block 0.
            norm_transpose(txt_t, 128, txtT, nc.vector.tensor_copy)
            norm_transpose(img_t, M_PER, imgT, nc.scalar.copy)
            itps = psum.tile([128, 512], F32, tag="mm", name="itps")
            for k in range(KC):
                nc.tensor.matmul(itps[:, 0:M_PER], lhsT=txtT[:, k, :],
                                 rhs=imgT[:, k, :], start=(k == 0),
                                 stop=(k == KC - 1))
            nc.scalar.activation(out=yit[:], in_=itps[:, 0:M_PER],
                                 func=AF.Copy, bias=float(bias),
                                 scale=float(t))

            for b in range(5, 13):
                prep_block_pe(b)

            def softplus_out(y_ap, d_out):
                el = small.tile([B, M_PER], F32, tag="el", name="el")
                nc.scalar.activation(out=el[:], in_=y_ap, func=AF.Exp)
                nc.vector.tensor_scalar_add(el[:], el[:], 1.0)
                nc.scalar.activation(out=el[:], in_=el[:], func=AF.Ln)
                nc.sync.dma_start(out=d_out.ap(), in_=el[:])

            # ---------- main loop: fp8 DoubleRow patch x concept -----------
            def main_pass(pt, extras=()):
                extras = dict(extras)
                base = pt * HALF
                for c in range(C):
                    ps4 = psum.tile([128, 4, 512], F32, tag="mm", name="ps4")
                    for kp in range(KC // 2):
                        for i in range(4):
                            nc.tensor.matmul(
                                ps4[:, i, 0:CHW],
                                lhsT=cT[:, 2 * kp:2 * kp + 2,
                                        c * 128:(c + 1) * 128],
                                rhs=rhs[:, 2 * kp:2 * kp + 2,
                                        base + i * CHW:base + (i + 1) * CHW,
                                        0],
                                start=(kp == 0), stop=(kp == KC // 2 - 1),
                                perf_mode=PM.DoubleRow)
                    nc.vector.reduce_max(
                        out=maxcol[:, c, pt * 8:pt * 8 + 8].rearrange(
                            "p (i m) -> p i m", i=4),
                        in_=ps4[:, :, 0:CHW].rearrange(
                            "p i (m n) -> p i m n", m=2),
                        axis=AX.X)
                    if c in extras:
                        extras[c]()
                for k in sorted(extras):
                    if k >= C:  # safety: C is data-dependent
                        extras[k]()

            ex0 = {c: (lambda b=13 + c: prep_block_xbar(b))
                   for c in range(12)}
            ex1 = {0: lambda: concept_chunk(0, 6),
                   1: lambda: concept_chunk(6, 12),
                   2: lambda: concept_chunk(12, C),
                   3: late_tables,
                   4: lambda: (nc.vector.tensor_mul(yit[:], yit[:], sign[:]),
                               softplus_out(yit[:], d_it))}
            main_pass(0, ex0)
            main_pass(1, ex1)

            # ---------- S = G_eff^T @ maxcol (bf16), losses ----------------
            sps = psum.tile([128, 512], F32, tag="mm", name="sps")
            for c in range(C):
                nc.tensor.matmul(sps[:, 0:M_PER],
                                 lhsT=GTb[:, c, :],
                                 rhs=maxcol[:, c, :], start=(c == 0),
                                 stop=(c == C - 1))

            yrc = small.tile([B, M_PER], F32, tag="y")
            # affine on DVE so the tail touches only the Exp/Ln ACT tables
            # (a third func would force 1.3us table reloads after the last
            # concept-norm Sqrt evictions)
            nc.vector.tensor_scalar(out=yrc[:], in0=sps[:, 0:M_PER],
                                    scalar1=float(t), scalar2=float(bias),
                                    op0=mybir.AluOpType.mult,
                                    op1=mybir.AluOpType.add)
            nc.vector.tensor_mul(yrc[:], yrc[:], sign[:])
            softplus_out(yrc[:], d_rc)

    nc.compile()
    return nc


def _install_trace_hook():
    """Register the axon NTFF profiling hook (missing from this image) so
    run_bass_kernel_spmd(trace=True) can capture HW exec time."""
    import contextlib
    import ctypes
    import types

    import concourse.bass_utils as bu

    if "antenv.axon_hooks" in sys.modules:
        return
    so_path = "/opt/axon/libaxon_pjrt.so"

    def _make_hook():
        lib = ctypes.CDLL(so_path)
        if not hasattr(lib, "axon_start_nrt_profile"):
            return None
        lib.axon_start_nrt_profile.argtypes = [ctypes.POINTER(ctypes.c_int64),
                                               ctypes.c_size_t]
        lib.axon_start_nrt_profile.restype = ctypes.c_int64
        lib.axon_stop_nrt_profile.argtypes = [ctypes.c_char_p]
        lib.axon_stop_nrt_profile.restype = ctypes.c_int64

        @contextlib.contextmanager
        def _hook(output_dir, device_ids):
            import jax
            jax.devices()
            if device_ids:
                ids = (ctypes.c_int64 * len(device_ids))(*device_ids)
                rc = lib.axon_start_nrt_profile(ids, len(device_ids))
            else:
                rc = lib.axon_start_nrt_profile(None, 0)
            if rc != 0:
                raise RuntimeError(f"axon_start_nrt_profile rc={rc}")
            try:
                yield
            finally:
                n = lib.axon_stop_nrt_profile(str(output_dir).encode())
                print(f"profile: {n} file(s) written to {output_dir}",
                      file=sys.stderr)

        return _hook

    mod = types.ModuleType("antenv.axon_hooks")
    mod.get_axon_ntff_profile_hook = _make_hook
    sys.modules["antenv.axon_hooks"] = mod
    bu.upload_artifacts = lambda tmpdir: tmpdir  # no S3 in this container


def _prepare(inputs):
    image_features = np.asarray(inputs["image_features"], np.float32)
    text_features = np.asarray(inputs["text_features"], np.float32)
    image_token_features = np.asarray(inputs["image_token_features"], np.float32)
    concept_text_features = np.asarray(inputs["concept_text_features"], np.float32)
    counts = np.asarray(inputs["concept_counts"]).astype(np.int64)
    t = float(np.exp(np.clip(np.float32(inputs["logit_scale"]), -10.0, 10.0)))
    bias = float(np.float32(inputs["logit_bias"]))

    # pack concepts: keep only w < counts[v]; pad rows with ones (zero weight)
    vidx = np.repeat(np.arange(B), counts)
    widx = np.concatenate([np.arange(c) for c in counts])
    P = len(vidx)
    C = math.ceil(P / 128)
    Ppad = C * 128
    cnat = np.ones((Ppad, D), np.float32)
    cnat[:P] = concept_text_features[vidx, widx]
    cnat8 = cnat.astype(FP8)
    cT = np.ascontiguousarray(cnat8.T).reshape(KC, 128, Ppad)

    G = np.zeros((Ppad, B), np.float32)
    G[np.arange(P), vidx] = 1.0 / counts[vidx]
    GT = G.reshape(C, 128, B)

    txt_bf = text_features.astype(BF16)
    ident = np.eye(128, dtype=BF16)

    in_maps = []
    for core in range(N_CORES):
        s = slice(core * M_PER, (core + 1) * M_PER)
        signneg = np.ones((B, M_PER), np.float32)
        for j in range(M_PER):
            signneg[core * M_PER + j, j] = -1.0
        pat = np.ones((NPAD, D), np.float32)
        pat[:NPF] = image_token_features[s].reshape(NPF, D)
        in_maps.append({
            "patches": pat.astype(BF16),
            "cT": cT,
            "cnat": cnat8,
            "GT": GT,
            "img": image_features[s].astype(BF16),
            "txt": txt_bf,
            "signneg": signneg,
            "ident": ident,
        })
    return in_maps, C, t, bias


def _run(inputs, trace=False, tmpdir=None):
    in_maps, C, t, bias = _prepare(inputs)
    key = (C, t, bias)
    if key not in _cache:
        _cache[key] = _build(C, t, bias)
    nc = _cache[key]
    kwargs = {}
    if trace:
        _install_trace_hook()
        kwargs = dict(trace=True, tmpdir=tmpdir)
    res = run_bass_kernel_spmd(nc, in_maps, core_ids=list(range(N_CORES)),
                               **kwargs)
    it_sum = sum(float(r["it_el"].astype(np.float64).sum()) for r in res.results)
    rc_sum = sum(float(r["rc_el"].astype(np.float64).sum()) for r in res.results)
    it_loss = it_sum / (B * B)
    rc_loss = rc_sum / (B * B)
    total = it_loss + 0.5 * rc_loss
    out = (np.float32(total), np.float32(it_loss), np.float32(rc_loss))
    return out, res


def kernel(**inputs):
    out, _ = _run(inputs)
    return out
